# revision 7
# baseline (speedup 1.0000x reference)
"""BiAttention Trainium2 kernel (8 NeuronCores, batch-parallel).

Problem (per batch element b, 8 of them -> one per core):
    A_proj = A @ W_A + b_A            [2048, 64]
    B_proj = B @ W_B + b_B            [2048, 64]
    S      = A_proj @ B_proj^T        [2048, 2048]
    A_star = softmax(S, axis=-1) @ B  [2048, 768]
    B_star = softmax(S, axis=0)^T @ A [2048, 768]

Key algebra used on-device (|S| < ~30, so exp(S) is safe in f32/bf16
without max-subtraction):
    E = exp(S)
    A_star = diag(1/rowsum(E)) . (E @ B)
    B_star = diag(1/colsum(E)) . (E^T @ A)
rowsum/colsum come for free from a ones-column in the moving operand.

E is never materialized in full: score panels are recomputed per
512-wide output stripe (K=64 contraction - cheap) directly from the
projections, exp'd into bf16 packs, and immediately consumed as the
stationary operand of the big matmuls.

v4 schedule notes:
  * casting loads write straight into the `aug` moving-operand tensors
    (no staging buffer); PE transposes read aug slices as stationary.
  * load order A0,A1 then B0..B7 then A2..A7.  The first work item
    (A_star stripe 0) is software-pipelined against B's ARRIVAL: as
    each B unit lands it is transposed, projected, scored against the
    stripe-0 A projection, exp'd, and immediately folded into two
    partially-accumulated psum blocks (ii=0,1; psum only fits two
    alongside the score/transpose ring).  The PE therefore does dense
    real work from ~15us on, which also ramps the clock gate early.
  * remaining prep (A units 2-7, later projections, score packs) is
    hooked between accum blocks of earlier work items.
"""

import sys

if "/opt/trn_rl_repo" not in sys.path:
    sys.path.insert(0, "/opt/trn_rl_repo")

import numpy as np
import ml_dtypes

import concourse.bass as bass
import concourse.mybir as mybir
import concourse.tile as tile
from concourse import bacc
from concourse.bass import ts
from concourse.bass_utils import run_bass_kernel_spmd

F32 = mybir.dt.float32
BF16 = mybir.dt.bfloat16
AF = mybir.ActivationFunctionType

L = 2048          # sequence length (both La and Lb)
D = 768           # model dim
H = 64            # projection dim
NT = L // 128     # 16 row/col tiles of 128
KD = D // 128     # 6 contraction tiles for the projections
NSUP = L // 512   # 4 supers (512-wide output stripes)
DP = D + 1        # moving operand width with the ones column

N_CORES = 8

_CACHE = {}

_IDENT = np.eye(128, dtype=ml_dtypes.bfloat16)


def _build():
    nc = bacc.Bacc("TRN2", target_bir_lowering=False, debug=False,
                   num_devices=N_CORES)
    A_d = nc.dram_tensor("A", [L, D], F32, kind="ExternalInput").ap()
    B_d = nc.dram_tensor("B", [L, D], F32, kind="ExternalInput").ap()
    WA_d = nc.dram_tensor("W_A", [D, H], F32, kind="ExternalInput").ap()
    WB_d = nc.dram_tensor("W_B", [D, H], F32, kind="ExternalInput").ap()
    bA_d = nc.dram_tensor("b_A", [H, 1], F32, kind="ExternalInput").ap()
    bB_d = nc.dram_tensor("b_B", [H, 1], F32, kind="ExternalInput").ap()
    ID_d = nc.dram_tensor("IDENT", [128, 128], BF16, kind="ExternalInput").ap()
    AS_d = nc.dram_tensor("A_star", [L, D], F32, kind="ExternalOutput").ap()
    BS_d = nc.dram_tensor("B_star", [L, D], F32, kind="ExternalOutput").ap()

    with tile.TileContext(nc) as tc:
        with (
            tc.tile_pool(name="mov", bufs=1) as pmov,
            tc.tile_pool(name="pack", bufs=18) as ppack,
            tc.tile_pool(name="outp", bufs=4) as pout,
            tc.tile_pool(name="psum", bufs=2, space="PSUM") as pps,
        ):
            ident = pmov.tile([128, 128], BF16, tag="ident", name="ident")
            warm = pmov.tile([128, 512], BF16, tag="warm", name="warm")

            dram = {"A": A_d, "B": B_d}
            aug = {}
            projT = {}
            mts = {}
            for side in ("A", "B"):
                aug[side] = pmov.tile([128, NT, DP], BF16, tag=f"aug{side}",
                                      name=f"{side}_aug")
                projT[side] = pmov.tile([128, L], BF16, tag=f"p{side}",
                                        name=f"{side}_projT")
                mts[side] = pmov.tile([128, NT * KD, 128], BF16,
                                      tag=f"t{side}", name=f"{side}_T")

            w_sb = {}
            b_sb = {}

            def load_weights():
                for side, (W_dram, b_dram) in (
                    ("B", (WB_d, bB_d)), ("A", (WA_d, bA_d))
                ):
                    wb = pmov.tile([128, KD, H], BF16, tag=f"w{side}",
                                   name=f"w{side}b")
                    nc.gpsimd.dma_start(
                        out=wb, in_=W_dram.rearrange("(k p) h -> p k h", p=128)
                    )
                    bt = pmov.tile([H, 1], F32, tag=f"b{side}",
                                   name=f"b{side}sb")
                    nc.scalar.dma_start(out=bt, in_=b_dram)
                    w_sb[side] = wb
                    b_sb[side] = bt

            def load_unit(side, u, split=False):
                # casting DMA f32 DRAM -> bf16 straight into aug (SWDGE)
                if split:
                    for t in range(2):
                        i = 2 * u + t
                        nc.gpsimd.dma_start(out=aug[side][:, i, 0:D],
                                            in_=dram[side][ts(i, 128), :])
                else:
                    nc.gpsimd.dma_start(
                        out=aug[side][:, 2 * u:2 * u + 2, 0:D],
                        in_=dram[side][u * 256:(u + 1) * 256, :].rearrange(
                            "(t p) d -> p t d", p=128
                        ),
                    )

            def trans_tile(side, i):
                # transpose tile i's 6 blocks on TensorE; vector moves the
                # result to mts (scalar stays free for activations/exps)
                ps = pps.tile([128, 1024], F32, tag="spack",
                              name=f"pstr{side}{i}")
                for j in range(KD):
                    nc.tensor.matmul(ps[:, ts(j, 128)],
                                     aug[side][:, i, ts(j, 128)],
                                     ident, start=True, stop=True)
                nc.vector.tensor_copy(
                    out=mts[side][:, i * KD:(i + 1) * KD, :],
                    in_=ps[:, 0:KD * 128],
                )

            def proj_cols(side, c0, ncols):
                # projT[h, c0:c0+ncols] (+bias, +dup into rows 64:128)
                mtv = mts[side].rearrange("p (i j) q -> p i j q", j=KD)
                i0 = c0 // 128
                nt = ncols // 128
                ps = pps.tile([128, 1024], F32, tag="spack",
                              name=f"psproj{side}{c0}")
                for k in range(KD):
                    nc.tensor.matmul(
                        ps[:H, 0:ncols],
                        w_sb[side][:, k, :],
                        mtv[:, i0:i0 + nt, k, :],
                        start=(k == 0), stop=(k == KD - 1),
                    )
                nc.scalar.activation(
                    out=projT[side][0:H, c0:c0 + ncols], in_=ps[:H, 0:ncols],
                    func=AF.Identity, bias=b_sb[side], scale=1.0,
                )
                nc.sync.dma_start(out=projT[side][H:128, c0:c0 + ncols],
                                  in_=projT[side][0:H, c0:c0 + ncols])

            # ---- prelude ----
            nc.vector.memset(warm, 0.0)
            for side in ("A", "B"):
                nc.vector.memset(aug[side][:, :, D:DP], 1.0)
            nc.sync.dma_start(out=ident, in_=ID_d)
            load_weights()
            load_unit("A", 0, split=True)
            load_unit("A", 1, split=True)
            for u in range(8):
                load_unit("B", u)
            for u in range(2, 8):
                load_unit("A", u)

            # clock-ramp warmup; all fillers precede the arrival-phase psum
            # allocs so their ring slot is never live
            wps = pps.tile([128, 1024], F32, tag="accum", name="warmps")

            def filler(n, width=512):
                for _ in range(n):
                    nc.tensor.matmul(wps[:, 0:width], warm[:, 0:128],
                                     warm[:, 0:width], start=True, stop=True)

            filler(10, 128)
            for i in range(4):
                trans_tile("A", i)
                filler(3)
            proj_cols("A", 0, 512)
            filler(3)

            # ---- main loop ----
            # dirn "A": A_star rows; panels E^T[t, s-stripe]
            #   (lhsT = B_projT tiles, rhs = A_projT stripe), moving = aug_B
            # dirn "B": B_star rows; panels E[s, t-stripe]
            #   (lhsT = A_projT tiles, rhs = B_projT stripe), moving = aug_A
            spec = {
                "A": (projT["B"], projT["A"], aug["B"], AS_d),
                "B": (projT["A"], projT["B"], aug["A"], BS_d),
            }
            pkts = {}

            def emit_pack_piece(dirn, u, jps):
                pT_l, pT_r, _, _ = spec[dirn]
                for jp in jps:
                    pkt = ppack.tile([128, 1024], BF16, tag="pack", bufs=18,
                                     name=f"pk{dirn}{u}{jp}")
                    ps = pps.tile([128, 1024], F32, tag="spack",
                                  name=f"pss{dirn}{u}{jp}")
                    for h2 in range(2):
                        j = jp * 2 + h2
                        base = h2 * 64
                        nc.tensor.matmul(
                            ps[:, ts(h2, 512)],
                            pT_l[base:base + H, ts(j, 128)],
                            pT_r[base:base + H, ts(u, 512)],
                            start=True, stop=True,
                            tile_position=(base, 0),
                        )
                    nc.scalar.activation(out=pkt, in_=ps, func=AF.Exp)
                    pkts[(dirn, u, jp)] = pkt

            def accum_js(dirn, u, ii, pa, js):
                _, _, mv, _ = spec[dirn]
                for j in js:
                    lhs = pkts[(dirn, u, j // 2)][
                        :, (j % 2) * 512 + ii * 128:(j % 2) * 512 + ii * 128 + 128]
                    # short mm first: the trailing 512-col mm covers the
                    # next tile's LDWEIGHTS pull-ahead window
                    nc.tensor.matmul(
                        pa[:, 512:DP], lhs, mv[:, j, 512:DP],
                        start=(j == 0), stop=(j == NT - 1),
                    )
                    nc.tensor.matmul(
                        pa[:, 0:512], lhs, mv[:, j, 0:512],
                        start=(j == 0), stop=(j == NT - 1),
                    )

            def norm_store(dirn, u, ii, pa, nsplit=1):
                _, _, _, out_d = spec[dirn]
                rinv = pout.tile([128, 1], F32, tag="rinv",
                                 name=f"ri{dirn}{u}{ii}")
                nc.vector.reciprocal(out=rinv, in_=pa[:, D:DP])
                ot = pout.tile([128, D], F32, tag="ot",
                               name=f"ot{dirn}{u}{ii}")
                w = D // nsplit
                for h in range(nsplit):
                    nc.vector.tensor_scalar_mul(ot[:, h * w:(h + 1) * w],
                                                pa[:, h * w:(h + 1) * w], rinv)
                    nc.sync.dma_start(
                        out=out_d[ts(u * 4 + ii, 128), h * w:(h + 1) * w],
                        in_=ot[:, h * w:(h + 1) * w],
                    )

            def accum_block(dirn, u, ii, nsplit=1):
                pa = pps.tile([128, 1024], F32, tag="accum",
                              name=f"pac{dirn}{u}{ii}")
                accum_js(dirn, u, ii, pa, range(NT))
                norm_store(dirn, u, ii, pa, nsplit)

            # -- arrival-pipelined first item: (A, 0) ii=0,1 --
            pa_arr = None
            for u in range(8):
                trans_tile("B", 2 * u)
                trans_tile("B", 2 * u + 1)
                if u == 0:
                    pa_arr = [
                        pps.tile([128, 1024], F32, tag="accum", name="paArr0"),
                        pps.tile([128, 1024], F32, tag="accum", name="paArr1"),
                    ]
                else:
                    for ii in (0, 1):
                        accum_js("A", 0, ii, pa_arr[ii],
                                 (2 * (u - 1), 2 * (u - 1) + 1))
                proj_cols("B", u * 256, 256)
                emit_pack_piece("A", 0, [u])
            for ii in (0, 1):
                accum_js("A", 0, ii, pa_arr[ii], (14, 15))
                norm_store("A", 0, ii, pa_arr[ii])

            def prep_a(u2):
                # transposes + projection cols for A units 2u2, 2u2+1
                def fn():
                    for i in range(4 * u2, 4 * u2 + 4):
                        trans_tile("A", i)
                    proj_cols("A", u2 * 512, 512)
                return fn

            def pack_fn(dirn, u, jps):
                return lambda: emit_pack_piece(dirn, u, jps)

            def emit_item(dirn, u, after, last=False):
                for ii in range(4):
                    accum_block(dirn, u, ii,
                                nsplit=2 if (last and ii == 3) else 1)
                    if after and ii in after:
                        for fn in after[ii]:
                            fn()

            accum_block("A", 0, 2)
            prep_a(1)()
            accum_block("A", 0, 3)
            emit_pack_piece("A", 1, range(8))
            emit_item("A", 1, {0: [prep_a(2)],
                               1: [pack_fn("A", 2, range(4))],
                               2: [pack_fn("A", 2, range(4, 8))]})
            emit_item("A", 2, {0: [prep_a(3)],
                               1: [pack_fn("A", 3, range(4))],
                               2: [pack_fn("A", 3, range(4, 8))]})
            emit_item("A", 3, {1: [pack_fn("B", 0, range(4))],
                               2: [pack_fn("B", 0, range(4, 8))]})
            emit_item("B", 0, {1: [pack_fn("B", 1, range(4))],
                               2: [pack_fn("B", 1, range(4, 8))]})
            emit_item("B", 1, {1: [pack_fn("B", 2, range(4))],
                               2: [pack_fn("B", 2, range(4, 8))]})
            emit_item("B", 2, {1: [pack_fn("B", 3, range(4))],
                               2: [pack_fn("B", 3, range(4, 8))]})
            emit_item("B", 3, None, last=True)

    nc.compile()
    return nc


def _get_nc():
    if "nc" not in _CACHE:
        _CACHE["nc"] = _build()
    return _CACHE["nc"]


def _run(inputs, trace=False):
    nc = _get_nc()
    A = np.ascontiguousarray(np.asarray(inputs["A"], dtype=np.float32))
    B = np.ascontiguousarray(np.asarray(inputs["B"], dtype=np.float32))
    W_A = np.ascontiguousarray(np.asarray(inputs["W_A"], dtype=np.float32))
    W_B = np.ascontiguousarray(np.asarray(inputs["W_B"], dtype=np.float32))
    b_A = np.asarray(inputs["b_A"], dtype=np.float32).reshape(H, 1)
    b_B = np.asarray(inputs["b_B"], dtype=np.float32).reshape(H, 1)
    in_maps = [
        {
            "A": A[c], "B": B[c],
            "W_A": W_A, "W_B": W_B,
            "b_A": b_A, "b_B": b_B,
            "IDENT": _IDENT,
        }
        for c in range(N_CORES)
    ]
    res = run_bass_kernel_spmd(nc, in_maps, list(range(N_CORES)), trace=trace)
    A_star = np.stack([res.results[c]["A_star"] for c in range(N_CORES)])
    B_star = np.stack([res.results[c]["B_star"] for c in range(N_CORES)])
    return A_star, B_star, res


def kernel(**inputs):
    A_star, B_star, _ = _run(inputs)
    return A_star, B_star
